# revision 2
# baseline (speedup 1.0000x reference)
"""BitLinear (1-bit packed weights) matmul kernel for 8 Trainium2 NeuronCores.

Computes out = x @ w.T where w[o, k] in {-1, +1} is unpacked from bytes
bp (one byte per int32 element, 8 weights per byte, MSB-first).

Strategy (tensor-parallel over out features, x replicated):
  - Each core owns OUT_F/8 = 1376 output features.
  - Identity: w = 2*b - 1 (b in {0,1})  =>  out = 2*(x @ b.T) - sum_k x~.
  - Bit-plane decomposition: k = 8j + p; byte bit index j_bit = 7 - p.
  - fp8 exponent-field unpack (1 DVE int8 op per plane): host pre-shifts
    the byte matrix (b<<4, b<<1, b>>2) so each weight bit can be isolated
    at an fp8 E4M3 exponent-bit position (4, 5 or 6) by a bitwise AND.
    The surviving single-bit pattern *is* an exact power of two
    c in {2^-5, 2^-3, 2} (TRN E4M3: bias 7, max normal 240). The 1/c
    normalization is folded into the host-side per-plane scaling of x.
  - Mixed precision (rel-err budget 2e-2; quantization measured 1.88e-2):
      planes 0..3 (16 of 32 k-tiles): x in E4M3, matmuls run as
        perf_mode=DoubleRow fp8 pairs (2 k-tiles per instruction,
        ~1.4x bf16 throughput at moving free dim 2x512);
      planes 4..7: x in bf16 (stationary) x fp8 weights (moving) --
        plain mode, same speed as bf16xbf16, no extra quantization.
  - The rowsum correction uses R~ = sum_k x~_k of the *quantized* x
    (not raw x): error becomes sum_k eps_k*w_k instead of picking up an
    extra (sum_k eps_k)^2 term -- ~sqrt(2) lower error for free.
  - Per psum tile [t=128, o<=512]: 8 DoubleRow + 16 plain matmuls,
    evict with ACT/DVE (scale=2, bias=-R~) to f32.

Host-side prep is layout/quantization only: per-plane pow2-scaled casts
of x, byte-matrix shifts of bp, rowsum of the quantized x.
"""

from contextlib import ExitStack

import numpy as np
import ml_dtypes

import concourse.bass as bass
import concourse.mybir as mybir
import concourse.tile as tile
from concourse.bass_utils import run_bass_kernel_spmd


def _ensure_axon_hooks_module():
    """concourse's trace path imports antenv.axon_hooks unconditionally when
    BASS_TRACE is set; some images lack it. Provide a stub so tracing
    degrades gracefully instead of crashing."""
    try:
        import antenv.axon_hooks  # noqa: F401
    except ImportError:
        import sys
        import types

        import antenv

        mod = types.ModuleType("antenv.axon_hooks")
        mod._hook = None

        def set_axon_ntff_profile_hook(h, _mod=mod):
            _mod._hook = h

        def get_axon_ntff_profile_hook(_mod=mod):
            return _mod._hook

        mod.set_axon_ntff_profile_hook = set_axon_ntff_profile_hook
        mod.get_axon_ntff_profile_hook = get_axon_ntff_profile_hook
        sys.modules["antenv.axon_hooks"] = mod
        antenv.axon_hooks = mod


_ensure_axon_hooks_module()

TOKENS, IN_F, OUT_F = 1024, 4096, 11008
N_CORES = 8
OS = OUT_F // N_CORES      # 1376 out features per core
J = IN_F // 8              # 512 packed bytes per out feature
JT = J // 128              # 4 j-tiles
TT = TOKENS // 128         # 8 token tiles
O_CHUNKS = [512, 512, 352]  # sums to OS
N_FP8_PLANES = 4           # planes 0..3 via fp8 DoubleRow pairs

# plane p uses byte bit j = 7 - p, shifted into an fp8 exponent-bit
# position by one of three host-prepared source arrays:
#   SA = byte << 4  (bits 0,1,2 -> positions 4,5,6)
#   SB = byte << 1  (bits 3,4,5 -> positions 4,5,6)
#   SC = byte >> 2  (bits 6,7   -> positions 4,5)
# single exponent bit at position 4/5/6 decodes to c = 2^-5 / 2^-3 / 2.
_PLANES = {
    0: ("SC", 1 << 5, 2.0 ** -3),   # j=7
    1: ("SC", 1 << 4, 2.0 ** -5),   # j=6
    2: ("SB", 1 << 6, 2.0),         # j=5
    3: ("SB", 1 << 5, 2.0 ** -3),   # j=4
    4: ("SB", 1 << 4, 2.0 ** -5),   # j=3
    5: ("SA", 1 << 6, 2.0),         # j=2
    6: ("SA", 1 << 5, 2.0 ** -3),   # j=1
    7: ("SA", 1 << 4, 2.0 ** -5),   # j=0
}

_CACHE: dict = {}

_MAX_WAITS = 1  # walrus codegen rejects instructions with more sem waits


def _legalize_waits(nc) -> int:
    """Split instructions carrying >_MAX_WAITS sem waits into preceding
    same-engine NoOps (Tile's tail drain aggregates one wait per live
    semaphore, which walrus codegen rejects)."""
    n_split = 0
    for fn in nc.m.functions:
        for bb in fn.blocks:
            insts = list(bb.instructions)
            out = []
            for inst in insts:
                si = getattr(inst, "sync_info", None)
                waits = list(si.on_wait) if (si is not None and si.on_wait) else []
                if len(waits) > _MAX_WAITS:
                    extra = waits[:-_MAX_WAITS]
                    keep = waits[-_MAX_WAITS:]
                    for i in range(0, len(extra), _MAX_WAITS):
                        chunk = extra[i:i + _MAX_WAITS]
                        out.append(mybir.InstNoOp(
                            name=f"{inst.name}_wsplit{i}",
                            engine=inst.engine,
                            ins=[],
                            outs=[],
                            sync_info=mybir.SyncInfo(on_wait=chunk, on_update=[]),
                        ))
                    si.on_wait = keep
                    n_split += 1
                out.append(inst)
            if len(out) != len(insts):
                bb.instructions[:] = out
    return n_split


def _build_module() -> bass.Bass:
    nc = bass.Bass(
        "TRN2",
        target_bir_lowering=False,
        debug=False,
        enable_asserts=False,
        num_devices=N_CORES,
    )
    # fp8 x pairs: [q=128, sub=128, tok=128] e4m3, sub = flat(jt, pairi, t, h):
    #   xr8[q, ((jt*2+pairi)*TT + t)*2 + h, tok]
    #     = e4m3(x[t*128+tok, 8*(jt*128+q) + (2*pairi+h)] / c_plane)
    xr8_d = nc.dram_tensor(
        "xr8", [128, 2 * JT * TT * 2, 128], mybir.dt.float8e4, kind="ExternalInput"
    ).ap()
    # bf16 x planes 4..7: [q=128, (pi, jt, t)*128 tok] bf16:
    #   xrb[q, ((pi*JT + jt)*TOKENS) + t*128 + tok] = bf16(x[.., k]/c), pi=p-4
    xrb_d = nc.dram_tensor(
        "xrb", [128, 4 * JT * TOKENS], mybir.dt.bfloat16, kind="ExternalInput"
    ).ap()
    # byte-shift sources: [q=128, (jt, o)] int8
    sa_d = nc.dram_tensor("sa", [128, JT * OS], mybir.dt.int8, kind="ExternalInput").ap()
    sb_d = nc.dram_tensor("sb", [128, JT * OS], mybir.dt.int8, kind="ExternalInput").ap()
    sc_d = nc.dram_tensor("sc", [128, JT * OS], mybir.dt.int8, kind="ExternalInput").ap()
    # nrs layout: [q=128, tt] f32: -R~[tt*128+q]
    nrs_d = nc.dram_tensor(
        "nrs", [128, TT], mybir.dt.float32, kind="ExternalInput"
    ).ap()
    out_d = nc.dram_tensor(
        "out", [TOKENS, OS], mybir.dt.float32, kind="ExternalOutput"
    ).ap()

    with ExitStack() as ctx:
        tc = ctx.enter_context(tile.TileContext(nc))
        sb = ctx.enter_context(tc.tile_pool(name="sb", bufs=1))
        wpool = ctx.enter_context(tc.tile_pool(name="wpool", bufs=8))
        # 10 output slots: evictions must not stall on out-DMA completion
        # receipts (~2.4us each) recycling slots.
        opool = ctx.enter_context(tc.tile_pool(name="opool", bufs=10))
        ps = ctx.enter_context(tc.tile_pool(name="ps", bufs=1, space="PSUM"))

        # Byte-source loads on the ACT HWDGE ring (SP ring is busy with x),
        # split per (o-chunk, j-tile) in consumption order so the first
        # unpack isn't gated on the full transfer.
        sa_sb = sb.tile([128, JT * OS], mybir.dt.int8, name="sa_sb")
        sb_sb = sb.tile([128, JT * OS], mybir.dt.int8, name="sb_sb")
        sc_sb = sb.tile([128, JT * OS], mybir.dt.int8, name="sc_sb")
        nrs_sb = sb.tile([128, TT], mybir.dt.float32, name="nrs_sb")
        o0 = 0
        for ci, oc in enumerate(O_CHUNKS):
            for jt in range(JT):
                sl = slice(jt * OS + o0, jt * OS + o0 + oc)
                nc.scalar.dma_start(out=sc_sb[:, sl], in_=sc_d[:, sl])
                nc.scalar.dma_start(out=sb_sb[:, sl], in_=sb_d[:, sl])
                nc.scalar.dma_start(out=sa_sb[:, sl], in_=sa_d[:, sl])
            if ci == 0:
                # needed only by evictions; don't delay the first unpack
                nc.scalar.dma_start(out=nrs_sb, in_=nrs_d)
            o0 += oc

        # Resident x (6 MB total), streamed in consumption order (jt outer)
        # so the first tiles land early.
        xr8_sb = sb.tile([128, 2 * JT * TT * 2, 128], mybir.dt.float8e4,
                         name="xr8_sb")
        xrb_sb = sb.tile([128, 4 * JT * TOKENS], mybir.dt.bfloat16, name="xrb_sb")
        for jt in range(JT):
            lo = jt * 2 * TT * 2
            nc.sync.dma_start(
                out=xr8_sb[:, lo:lo + 2 * TT * 2, :],
                in_=xr8_d[:, lo:lo + 2 * TT * 2, :],
            )
            for pi in range(4):
                xlo = (pi * JT + jt) * TOKENS
                nc.sync.dma_start(
                    out=xrb_sb[:, xlo:xlo + TOKENS], in_=xrb_d[:, xlo:xlo + TOKENS]
                )

        # PE prewarm: dummy matmuls on memset tiles while the first byte
        # source is still in flight, so real MMs start at HAM 8/8 (2.4 GHz).
        warm_a = sb.tile([128, 128], mybir.dt.bfloat16, name="warm_a")
        nc.gpsimd.memset(warm_a, 0.0)
        warm_b = sb.tile([128, 512], mybir.dt.bfloat16, name="warm_b")
        nc.gpsimd.memset(warm_b, 0.0)
        warm_ps = ps.tile([128, 512], mybir.dt.float32, name="warm_ps", tag="ps0")
        for i in range(8):
            nc.tensor.matmul(
                warm_ps, lhsT=warm_a, rhs=warm_b,
                start=(i == 0), stop=(i == 7),
            )

        def evict(t, oc, o0, pst):
            # out = 2*psum - R~: alternate ACT/DVE so the eviction
            # chain keeps pace with PE's PSUM-bank reuse; out-DMAs issue
            # on both HWDGE rings.
            ot = opool.tile([128, 512], mybir.dt.float32, name="ot", tag="ot")
            if t % 2 == 0:
                nc.scalar.activation(
                    ot[:, :oc],
                    pst[:, :oc],
                    mybir.ActivationFunctionType.Identity,
                    bias=nrs_sb[:, t:t + 1],
                    scale=2.0,
                )
            else:
                nc.vector.tensor_scalar(
                    out=ot[:, :oc],
                    in0=pst[:, :oc],
                    scalar1=2.0,
                    scalar2=nrs_sb[:, t:t + 1],
                    op0=mybir.AluOpType.mult,
                    op1=mybir.AluOpType.add,
                )
            eng = nc.sync if t % 2 == 0 else nc.scalar
            eng.dma_start(
                out=out_d[t * 128:(t + 1) * 128, o0:o0 + oc], in_=ot[:, :oc]
            )

        srcs = {}

        def unpack8(src_name, mask, dst_ap, jt, o0, oc):
            src = srcs[src_name]
            nc.vector.tensor_scalar(
                out=dst_ap.bitcast(mybir.dt.int8),
                in0=src[:, jt * OS + o0: jt * OS + o0 + oc].bitcast(mybir.dt.int8),
                scalar1=mask,
                scalar2=None,
                op0=mybir.AluOpType.bitwise_and,
            )

        srcs = {"SA": sa_sb, "SB": sb_sb, "SC": sc_sb}

        # Per-jt unit order: DR pair, 2 plain, DR pair, 2 plain -- spreads
        # the 256-col DoubleRow LDWEIGHTS between cheaper 128-col loads.
        UNITS = []
        for jt in range(JT):
            UNITS.append(("pair", jt, 0))
            UNITS.append(("one", jt, 4))
            UNITS.append(("one", jt, 5))
            UNITS.append(("pair", jt, 1))
            UNITS.append(("one", jt, 6))
            UNITS.append(("one", jt, 7))

        o0 = 0
        for ci, oc in enumerate(O_CHUNKS):
            # For the final chunk, split token tiles into two groups so the
            # first group's evictions/stores hide under the second group's
            # matmuls (shorter post-MM tail). Costs one extra unpack pass.
            t_groups = [range(TT)] if ci < len(O_CHUNKS) - 1 else [
                range(0, 6), range(6, TT)
            ]
            psts = [
                ps.tile([128, 512], mybir.dt.float32, name=f"ps{i}", tag=f"ps{i}")
                for i in range(TT)
            ]
            for tg in t_groups:
                for ui, (kind, jt, pp) in enumerate(UNITS):
                    first = ui == 0
                    last = ui == len(UNITS) - 1
                    if kind == "pair":
                        wp8 = wpool.tile(
                            [128, 2, 512], mybir.dt.float8e4, name="wp8", tag="wp"
                        )
                        for h in range(2):
                            sname, mask, _c = _PLANES[2 * pp + h]
                            unpack8(sname, mask, wp8[:, h, :oc], jt, o0, oc)
                        for t in tg:
                            s = ((jt * 2 + pp) * TT + t) * 2
                            nc.tensor.matmul(
                                psts[t][:, :oc],
                                lhsT=xr8_sb[:, s:s + 2, :],
                                rhs=wp8[:, :, :oc],
                                start=first,
                                stop=last,
                                perf_mode=mybir.MatmulPerfMode.DoubleRow,
                            )
                    else:
                        sname, mask, _c = _PLANES[pp]
                        wp = wpool.tile(
                            [128, 512], mybir.dt.float8e4, name="wp", tag="wp"
                        )
                        unpack8(sname, mask, wp[:, :oc], jt, o0, oc)
                        for t in tg:
                            lo = ((pp - 4) * JT + jt) * TOKENS + t * 128
                            nc.tensor.matmul(
                                psts[t][:, :oc],
                                lhsT=xrb_sb[:, lo:lo + 128],
                                rhs=wp[:, :oc],
                                start=first,
                                stop=last,
                            )
                for t in tg:
                    evict(t, oc, o0, psts[t])
            o0 += oc
    _legalize_waits(nc)
    return nc


def _prep_inputs(x: np.ndarray, bp: np.ndarray):
    x = np.ascontiguousarray(x, dtype=np.float32)
    # xt[jt, q, p, t] = x[t, 8*(jt*128+q)+p]
    xt = np.ascontiguousarray(x.T).reshape(JT, 128, 8, TOKENS)

    xtilde_sum = np.zeros(TOKENS, dtype=np.float64)

    # fp8 planes 0..3 -> xr8 [128, sub, 128]
    xr8 = np.zeros((128, 2 * JT * TT * 2, 128), dtype=ml_dtypes.float8_e4m3)
    for p in range(N_FP8_PLANES):
        _s, _m, c = _PLANES[p]
        q8 = (xt[:, :, p, :] / np.float32(c)).astype(ml_dtypes.float8_e4m3)
        # q8[jt, q, t*128+tok]; device value = c * q8
        xtilde_sum += (q8.astype(np.float64) * c).sum(axis=(0, 1))
        jtv = np.arange(JT)[:, None]
        pairi, h = divmod(p, 2)
        subs = ((jtv * 2 + pairi) * TT + np.arange(TT)[None, :]) * 2 + h  # [JT, TT]
        q8r = q8.reshape(JT, 128, TT, 128)
        for jt in range(JT):
            for t in range(TT):
                xr8[:, subs[jt, t], :] = q8r[jt, :, t, :]

    # bf16 planes 4..7 -> xrb [128, (pi, jt, t)*tok]
    xrb = np.empty((128, 4 * JT * TOKENS), dtype=ml_dtypes.bfloat16)
    for p in range(N_FP8_PLANES, 8):
        _s, _m, c = _PLANES[p]
        qb = (xt[:, :, p, :] / np.float32(c)).astype(ml_dtypes.bfloat16)
        xtilde_sum += (qb.astype(np.float64) * c).sum(axis=(0, 1))
        pi = p - 4
        for jt in range(JT):
            lo = (pi * JT + jt) * TOKENS
            xrb[:, lo:lo + TOKENS] = qb[jt]

    nrs = np.ascontiguousarray(
        (-xtilde_sum).astype(np.float32).reshape(TT, 128).T
    )

    # bytes matrix [OUT_F, J] -> [q=128, jt, o] shifted copies
    bytes_m = bp.reshape(OUT_F, J).astype(np.uint8)
    bph = np.ascontiguousarray(
        bytes_m.T.reshape(JT, 128, OUT_F).transpose(1, 0, 2)
    )  # [128, JT, OUT_F]
    sa = ((bph.astype(np.uint16) << 4) & 0xFF).astype(np.uint8).view(np.int8)
    sbs = ((bph.astype(np.uint16) << 1) & 0xFF).astype(np.uint8).view(np.int8)
    sc = (bph >> 2).view(np.int8)

    in_maps = []
    for cidx in range(N_CORES):
        sl = slice(cidx * OS, (cidx + 1) * OS)
        in_maps.append({
            "xr8": xr8,
            "xrb": xrb,
            "sa": np.ascontiguousarray(sa[:, :, sl]).reshape(128, JT * OS),
            "sb": np.ascontiguousarray(sbs[:, :, sl]).reshape(128, JT * OS),
            "sc": np.ascontiguousarray(sc[:, :, sl]).reshape(128, JT * OS),
            "nrs": nrs,
        })
    return in_maps


def _run(x: np.ndarray, bp: np.ndarray, **spmd_kwargs):
    if "nc" not in _CACHE:
        _CACHE["nc"] = _build_module()
    nc = _CACHE["nc"]
    in_maps = _prep_inputs(x, bp)
    res = run_bass_kernel_spmd(
        nc, in_maps, core_ids=list(range(N_CORES)), **spmd_kwargs
    )
    out = np.concatenate([r["out"] for r in res.results], axis=1)
    return out, res


def _host_reference(x: np.ndarray, bp: np.ndarray) -> np.ndarray:
    # Safety net for inputs outside the fast path's envelope.
    shifts = np.arange(7, -1, -1)
    bits = (bp.astype(np.int64)[:, None] >> shifts) & 1
    w = bits.reshape(OUT_F, IN_F).astype(np.float32) * 2 - 1
    return (x @ w.T).astype(np.float32)


def kernel(x: np.ndarray, bp: np.ndarray) -> np.ndarray:
    x = np.asarray(x, dtype=np.float32)
    bp = np.asarray(bp)
    # fp8 planes scale x by up to 2^5; |x| must stay below the TRN E4M3
    # max normal (240) / 32 = 7.5. Standard-normal inputs sit near 5.1.
    if (not np.isfinite(x).all()) or np.abs(x).max() >= 7.0 \
            or bp.min() < 0 or bp.max() > 255:
        return _host_reference(x, bp)
    out, _ = _run(x, bp)
    return out


if __name__ == "__main__":
    rng = np.random.default_rng(0)
    x = rng.standard_normal((TOKENS, IN_F), dtype=np.float32)
    bp = rng.integers(0, 256, (OUT_F * IN_F // 8,), dtype=np.int32)
    out = kernel(x, bp)
    ref = _host_reference(x, bp)
    rel = np.linalg.norm(out - ref) / np.linalg.norm(ref)
    print("self-check rel err:", rel)


# revision 11
# speedup vs baseline: 1.0064x; 1.0064x over previous
"""BitLinear (1-bit packed weights) matmul kernel for 8 Trainium2 NeuronCores.

Computes out = x @ w.T where w[o, k] in {-1, +1} is unpacked from bytes
bp (one byte per int32 element, 8 weights per byte, MSB-first).

Strategy (tensor-parallel over out features, x replicated):
  - Each core owns OUT_F/8 = 1376 output features.
  - Identity: w = 2*b - 1 (b in {0,1})  =>  out = 2*(x @ b.T) - sum_k x~.
  - Bit-plane decomposition: k = 8j + p; byte bit index j_bit = 7 - p.
  - fp8 exponent-field unpack (1 DVE int8 op per plane): host pre-shifts
    the byte matrix (b<<4, b<<1, b>>2) so each weight bit can be isolated
    at an fp8 E4M3 exponent-bit position (4, 5 or 6) by a bitwise AND.
    The surviving single-bit pattern *is* an exact power of two
    c in {2^-5, 2^-3, 2} (TRN E4M3: bias 7, max normal 240). The 1/c
    normalization is folded into the host-side per-plane scaling of x.
  - Mixed precision (rel-err budget 2e-2; quantization measured 1.88e-2):
      planes 0..3 (16 of 32 k-tiles): x in E4M3, matmuls run as
        perf_mode=DoubleRow fp8 pairs (2 k-tiles per instruction,
        ~1.4x bf16 throughput at moving free dim 2x512);
      planes 4..7: x in bf16 (stationary) x fp8 weights (moving) --
        plain mode, same speed as bf16xbf16, no extra quantization.
  - The rowsum correction uses R~ = sum_k x~_k of the *quantized* x
    (not raw x): error becomes sum_k eps_k*w_k instead of picking up an
    extra (sum_k eps_k)^2 term -- ~sqrt(2) lower error for free.
  - Per psum tile [t=128, o<=512]: 8 DoubleRow + 16 plain matmuls,
    evict with ACT/DVE (scale=2, bias=-R~) to f32.

Host-side prep is layout/quantization only: per-plane pow2-scaled casts
of x, byte-matrix shifts of bp, rowsum of the quantized x.
"""

from contextlib import ExitStack

import numpy as np
import ml_dtypes

import concourse.bass as bass
import concourse.mybir as mybir
import concourse.tile as tile
from concourse.bass_utils import run_bass_kernel_spmd


def _ensure_axon_hooks_module():
    """concourse's trace path imports antenv.axon_hooks unconditionally when
    BASS_TRACE is set; some images lack it. Provide a stub so tracing
    degrades gracefully instead of crashing."""
    try:
        import antenv.axon_hooks  # noqa: F401
    except ImportError:
        import sys
        import types

        import antenv

        mod = types.ModuleType("antenv.axon_hooks")
        mod._hook = None

        def set_axon_ntff_profile_hook(h, _mod=mod):
            _mod._hook = h

        def get_axon_ntff_profile_hook(_mod=mod):
            return _mod._hook

        mod.set_axon_ntff_profile_hook = set_axon_ntff_profile_hook
        mod.get_axon_ntff_profile_hook = get_axon_ntff_profile_hook
        sys.modules["antenv.axon_hooks"] = mod
        antenv.axon_hooks = mod


_ensure_axon_hooks_module()

TOKENS, IN_F, OUT_F = 1024, 4096, 11008
N_CORES = 8
OS = OUT_F // N_CORES      # 1376 out features per core
J = IN_F // 8              # 512 packed bytes per out feature
JT = J // 128              # 4 j-tiles
TT = TOKENS // 128         # 8 token tiles
O_CHUNKS = [512, 512, 352]  # sums to OS
N_FP8_PLANES = 4           # planes 0..3 via fp8 DoubleRow pairs

# plane p uses byte bit j = 7 - p, shifted into an fp8 exponent-bit
# position by one of three host-prepared source arrays:
#   SA = byte << 4  (bits 0,1,2 -> positions 4,5,6)
#   SB = byte << 1  (bits 3,4,5 -> positions 4,5,6)
#   SC = byte >> 2  (bits 6,7   -> positions 4,5)
# single exponent bit at position 4/5/6 decodes to c = 2^-5 / 2^-3 / 2.
_PLANES = {
    0: ("SC", 1 << 5, 2.0 ** -3),   # j=7
    1: ("SC", 1 << 4, 2.0 ** -5),   # j=6
    2: ("SB", 1 << 6, 2.0),         # j=5
    3: ("SB", 1 << 5, 2.0 ** -3),   # j=4
    4: ("SB", 1 << 4, 2.0 ** -5),   # j=3
    5: ("SA", 1 << 6, 2.0),         # j=2
    6: ("SA", 1 << 5, 2.0 ** -3),   # j=1
    7: ("SA", 1 << 4, 2.0 ** -5),   # j=0
}

_CACHE: dict = {}

_MAX_WAITS = 1  # walrus codegen rejects instructions with more sem waits


def _legalize_waits(nc) -> int:
    """Split instructions carrying >_MAX_WAITS sem waits into preceding
    same-engine NoOps (Tile's tail drain aggregates one wait per live
    semaphore, which walrus codegen rejects)."""
    n_split = 0
    for fn in nc.m.functions:
        for bb in fn.blocks:
            insts = list(bb.instructions)
            out = []
            for inst in insts:
                si = getattr(inst, "sync_info", None)
                waits = list(si.on_wait) if (si is not None and si.on_wait) else []
                if len(waits) > _MAX_WAITS:
                    extra = waits[:-_MAX_WAITS]
                    keep = waits[-_MAX_WAITS:]
                    for i in range(0, len(extra), _MAX_WAITS):
                        chunk = extra[i:i + _MAX_WAITS]
                        out.append(mybir.InstNoOp(
                            name=f"{inst.name}_wsplit{i}",
                            engine=inst.engine,
                            ins=[],
                            outs=[],
                            sync_info=mybir.SyncInfo(on_wait=chunk, on_update=[]),
                        ))
                    si.on_wait = keep
                    n_split += 1
                out.append(inst)
            if len(out) != len(insts):
                bb.instructions[:] = out
    return n_split


def _build_module() -> bass.Bass:
    nc = bass.Bass(
        "TRN2",
        target_bir_lowering=False,
        debug=False,
        enable_asserts=False,
        num_devices=N_CORES,
    )
    # fp8 x pairs: [q=128, sub=128, tok=128] e4m3, sub = flat(jt, pairi, t, h):
    #   xr8[q, ((jt*2+pairi)*TT + t)*2 + h, tok]
    #     = e4m3(x[t*128+tok, 8*(jt*128+q) + (2*pairi+h)] / c_plane)
    xr8_d = nc.dram_tensor(
        "xr8", [128, 2 * JT * TT * 2, 128], mybir.dt.float8e4, kind="ExternalInput"
    ).ap()
    # bf16 x planes 4..7: [q=128, (jt, pi, t)*128 tok] bf16 (jt-major so the
    # per-jt working set is contiguous):
    #   xrb[q, ((jt*4 + pi)*TOKENS) + t*128 + tok] = bf16(x[.., k]/c), pi=p-4
    xrb_d = nc.dram_tensor(
        "xrb", [128, 4 * JT * TOKENS], mybir.dt.bfloat16, kind="ExternalInput"
    ).ap()
    # byte-shift sources: [q=128, (chunk, jt, o)] int8, chunk-major so each
    # o-chunk's working set is one contiguous DMA
    sa_d = nc.dram_tensor("sa", [128, JT * OS], mybir.dt.int8, kind="ExternalInput").ap()
    sb_d = nc.dram_tensor("sb", [128, JT * OS], mybir.dt.int8, kind="ExternalInput").ap()
    sc_d = nc.dram_tensor("sc", [128, JT * OS], mybir.dt.int8, kind="ExternalInput").ap()
    CHUNK_OFF = [0]
    for _oc in O_CHUNKS[:-1]:
        CHUNK_OFF.append(CHUNK_OFF[-1] + JT * _oc)
    # nrs layout: [q=128, tt] f32: -R~[tt*128+q]
    nrs_d = nc.dram_tensor(
        "nrs", [128, TT], mybir.dt.float32, kind="ExternalInput"
    ).ap()
    out_d = nc.dram_tensor(
        "out", [TOKENS, OS], mybir.dt.float32, kind="ExternalOutput"
    ).ap()

    with ExitStack() as ctx:
        tc = ctx.enter_context(tile.TileContext(nc))
        sb = ctx.enter_context(tc.tile_pool(name="sb", bufs=1))
        wpool = ctx.enter_context(tc.tile_pool(name="wpool", bufs=8))
        # 8 output slots: evictions must not stall on out-DMA completion
        # receipts (~2.4us each) recycling slots.
        opool = ctx.enter_context(tc.tile_pool(name="opool", bufs=8))
        ps = ctx.enter_context(tc.tile_pool(name="ps", bufs=1, space="PSUM"))

        # Byte-source loads on the ACT HWDGE ring (SP ring is busy with x):
        # one DMA per (array, o-chunk) thanks to the chunk-major layout;
        # SC first (the first DR pair unpacks from it).
        sa_sb = sb.tile([128, JT * OS], mybir.dt.int8, name="sa_sb")
        sb_sb = sb.tile([128, JT * OS], mybir.dt.int8, name="sb_sb")
        sc_sb = sb.tile([128, JT * OS], mybir.dt.int8, name="sc_sb")
        nrs_sb = sb.tile([128, TT], mybir.dt.float32, name="nrs_sb")
        for ci, oc in enumerate(O_CHUNKS):
            sl = slice(CHUNK_OFF[ci], CHUNK_OFF[ci] + JT * oc)
            nc.scalar.dma_start(out=sc_sb[:, sl], in_=sc_d[:, sl])
            nc.scalar.dma_start(out=sb_sb[:, sl], in_=sb_d[:, sl])
            nc.scalar.dma_start(out=sa_sb[:, sl], in_=sa_d[:, sl])
            if ci == 0:
                # needed only by evictions; don't delay the first unpack
                nc.scalar.dma_start(out=nrs_sb, in_=nrs_d)

        # Resident x (6 MB total), streamed in consumption order (jt outer,
        # DR pairs before plain planes); jt0 split fine so the first MMs
        # aren't gated on a large transfer.
        xr8_sb = sb.tile([128, 2 * JT * TT * 2, 128], mybir.dt.float8e4,
                         name="xr8_sb")
        xrb_sb = sb.tile([128, 4 * JT * TOKENS], mybir.dt.bfloat16, name="xrb_sb")
        for jt in range(JT):
            lo = jt * 2 * TT * 2
            if jt == 0:
                for pp in range(2):
                    nc.sync.dma_start(
                        out=xr8_sb[:, lo + pp * 16:lo + pp * 16 + 16, :],
                        in_=xr8_d[:, lo + pp * 16:lo + pp * 16 + 16, :],
                    )
                for pi in range(4):
                    xlo = (jt * 4 + pi) * TOKENS
                    nc.sync.dma_start(
                        out=xrb_sb[:, xlo:xlo + TOKENS],
                        in_=xrb_d[:, xlo:xlo + TOKENS],
                    )
            else:
                nc.sync.dma_start(
                    out=xr8_sb[:, lo:lo + 2 * TT * 2, :],
                    in_=xr8_d[:, lo:lo + 2 * TT * 2, :],
                )
                xlo = jt * 4 * TOKENS
                nc.sync.dma_start(
                    out=xrb_sb[:, xlo:xlo + 4 * TOKENS],
                    in_=xrb_d[:, xlo:xlo + 4 * TOKENS],
                )

        # PE prewarm: dummy matmuls on memset tiles while the first byte
        # source is still in flight, so real MMs start at HAM 8/8 (2.4 GHz).
        # 3 MMs (~1.9us cold) bridge until the first data lands; tag ps7 so
        # the bank conflicts with the *last* real start-MM, not the first.
        warm_a = sb.tile([128, 128], mybir.dt.bfloat16, name="warm_a")
        nc.gpsimd.memset(warm_a, 0.0)
        warm_b = sb.tile([128, 512], mybir.dt.bfloat16, name="warm_b")
        nc.gpsimd.memset(warm_b, 0.0)
        warm_ps = ps.tile([128, 512], mybir.dt.float32, name="warm_ps", tag="ps7")
        for i in range(3):
            nc.tensor.matmul(
                warm_ps, lhsT=warm_a, rhs=warm_b,
                start=(i == 0), stop=(i == 2),
            )

        def evict(t, oc, o0, pst):
            # out = 2*psum - R~: alternate ACT/DVE so the eviction
            # chain keeps pace with PE's PSUM-bank reuse; out-DMAs issue
            # on both HWDGE rings.
            ot = opool.tile([128, 512], mybir.dt.float32, name="ot", tag="ot")
            if t % 2 == 0:
                nc.scalar.activation(
                    ot[:, :oc],
                    pst[:, :oc],
                    mybir.ActivationFunctionType.Identity,
                    bias=nrs_sb[:, t:t + 1],
                    scale=2.0,
                )
            else:
                nc.vector.tensor_scalar(
                    out=ot[:, :oc],
                    in0=pst[:, :oc],
                    scalar1=2.0,
                    scalar2=nrs_sb[:, t:t + 1],
                    op0=mybir.AluOpType.mult,
                    op1=mybir.AluOpType.add,
                )
            eng = nc.sync if t % 2 == 0 else nc.scalar
            eng.dma_start(
                out=out_d[t * 128:(t + 1) * 128, o0:o0 + oc], in_=ot[:, :oc]
            )

        srcs = {"SA": sa_sb, "SB": sb_sb, "SC": sc_sb}

        def unpack8(src_name, mask, dst_ap, ci, jt, oc):
            src = srcs[src_name]
            lo = CHUNK_OFF[ci] + jt * oc
            nc.vector.tensor_scalar(
                out=dst_ap.bitcast(mybir.dt.int8),
                in0=src[:, lo:lo + oc].bitcast(mybir.dt.int8),
                scalar1=mask,
                scalar2=None,
                op0=mybir.AluOpType.bitwise_and,
            )

        # Per-jt unit order: DR pair, 2 plain, DR pair, 2 plain -- spreads
        # the 256-col DoubleRow LDWEIGHTS between cheaper 128-col loads.
        UNITS = []
        for jt in range(JT):
            UNITS.append(("pair", jt, 0))
            UNITS.append(("one", jt, 4))
            UNITS.append(("one", jt, 5))
            UNITS.append(("pair", jt, 1))
            UNITS.append(("one", jt, 6))
            UNITS.append(("one", jt, 7))

        o0 = 0
        for ci, oc in enumerate(O_CHUNKS):
            # For the final chunk, split token tiles into two groups so the
            # first group's evictions/stores hide under the second group's
            # matmuls (shorter post-MM tail). Costs one extra unpack pass.
            t_groups = [range(TT)] if ci < len(O_CHUNKS) - 1 else [
                range(0, 6), range(6, TT)
            ]
            psts = [
                ps.tile([128, 512], mybir.dt.float32, name=f"ps{i}", tag=f"ps{i}")
                for i in range(TT)
            ]
            for tg in t_groups:
                for ui, (kind, jt, pp) in enumerate(UNITS):
                    first = ui == 0
                    last = ui == len(UNITS) - 1
                    if kind == "pair":
                        wp8 = wpool.tile(
                            [128, 2, 512], mybir.dt.float8e4, name="wp8", tag="wp"
                        )
                        for h in range(2):
                            sname, mask, _c = _PLANES[2 * pp + h]
                            unpack8(sname, mask, wp8[:, h, :oc], ci, jt, oc)
                        for t in tg:
                            s = ((jt * 2 + pp) * TT + t) * 2
                            nc.tensor.matmul(
                                psts[t][:, :oc],
                                lhsT=xr8_sb[:, s:s + 2, :],
                                rhs=wp8[:, :, :oc],
                                start=first,
                                stop=last,
                                perf_mode=mybir.MatmulPerfMode.DoubleRow,
                            )
                    else:
                        sname, mask, _c = _PLANES[pp]
                        wp = wpool.tile(
                            [128, 512], mybir.dt.float8e4, name="wp", tag="wp"
                        )
                        unpack8(sname, mask, wp[:, :oc], ci, jt, oc)
                        for t in tg:
                            lo = (jt * 4 + (pp - 4)) * TOKENS + t * 128
                            nc.tensor.matmul(
                                psts[t][:, :oc],
                                lhsT=xrb_sb[:, lo:lo + 128],
                                rhs=wp[:, :oc],
                                start=first,
                                stop=last,
                            )
                for t in tg:
                    evict(t, oc, o0, psts[t])
            o0 += oc
    _legalize_waits(nc)
    return nc


def _prep_inputs(x: np.ndarray, bp: np.ndarray):
    x = np.ascontiguousarray(x, dtype=np.float32)
    # xt[jt, q, p, t] = x[t, 8*(jt*128+q)+p]
    xt = np.ascontiguousarray(x.T).reshape(JT, 128, 8, TOKENS)

    xtilde_sum = np.zeros(TOKENS, dtype=np.float64)

    # fp8 planes 0..3 -> xr8 [128, sub, 128]
    xr8 = np.zeros((128, 2 * JT * TT * 2, 128), dtype=ml_dtypes.float8_e4m3)
    for p in range(N_FP8_PLANES):
        _s, _m, c = _PLANES[p]
        q8 = (xt[:, :, p, :] / np.float32(c)).astype(ml_dtypes.float8_e4m3)
        # q8[jt, q, t*128+tok]; device value = c * q8
        xtilde_sum += (q8.astype(np.float64) * c).sum(axis=(0, 1))
        jtv = np.arange(JT)[:, None]
        pairi, h = divmod(p, 2)
        subs = ((jtv * 2 + pairi) * TT + np.arange(TT)[None, :]) * 2 + h  # [JT, TT]
        q8r = q8.reshape(JT, 128, TT, 128)
        for jt in range(JT):
            for t in range(TT):
                xr8[:, subs[jt, t], :] = q8r[jt, :, t, :]

    # bf16 planes 4..7 -> xrb [128, (jt, pi, t)*tok]
    xrb = np.empty((128, 4 * JT * TOKENS), dtype=ml_dtypes.bfloat16)
    for p in range(N_FP8_PLANES, 8):
        _s, _m, c = _PLANES[p]
        qb = (xt[:, :, p, :] / np.float32(c)).astype(ml_dtypes.bfloat16)
        xtilde_sum += (qb.astype(np.float64) * c).sum(axis=(0, 1))
        pi = p - 4
        for jt in range(JT):
            lo = (jt * 4 + pi) * TOKENS
            xrb[:, lo:lo + TOKENS] = qb[jt]

    nrs = np.ascontiguousarray(
        (-xtilde_sum).astype(np.float32).reshape(TT, 128).T
    )

    # bytes matrix [OUT_F, J] -> [q=128, jt, o] shifted copies
    bytes_m = bp.reshape(OUT_F, J).astype(np.uint8)
    bph = np.ascontiguousarray(
        bytes_m.T.reshape(JT, 128, OUT_F).transpose(1, 0, 2)
    )  # [128, JT, OUT_F]
    sa = ((bph.astype(np.uint16) << 4) & 0xFF).astype(np.uint8).view(np.int8)
    sbs = ((bph.astype(np.uint16) << 1) & 0xFF).astype(np.uint8).view(np.int8)
    sc = (bph >> 2).view(np.int8)

    def chunk_major(arr, sl):
        # [128, JT, OS-slice] -> [128, (chunk, jt, o_in_chunk)]
        a = arr[:, :, sl]
        parts = []
        o0 = 0
        for oc in O_CHUNKS:
            parts.append(a[:, :, o0:o0 + oc].reshape(128, JT * oc))
            o0 += oc
        return np.ascontiguousarray(np.concatenate(parts, axis=1))

    in_maps = []
    for cidx in range(N_CORES):
        sl = slice(cidx * OS, (cidx + 1) * OS)
        in_maps.append({
            "xr8": xr8,
            "xrb": xrb,
            "sa": chunk_major(sa, sl),
            "sb": chunk_major(sbs, sl),
            "sc": chunk_major(sc, sl),
            "nrs": nrs,
        })
    return in_maps


def _run(x: np.ndarray, bp: np.ndarray, **spmd_kwargs):
    if "nc" not in _CACHE:
        _CACHE["nc"] = _build_module()
    nc = _CACHE["nc"]
    in_maps = _prep_inputs(x, bp)
    res = run_bass_kernel_spmd(
        nc, in_maps, core_ids=list(range(N_CORES)), **spmd_kwargs
    )
    out = np.concatenate([r["out"] for r in res.results], axis=1)
    return out, res


def _host_reference(x: np.ndarray, bp: np.ndarray) -> np.ndarray:
    # Safety net for inputs outside the fast path's envelope.
    shifts = np.arange(7, -1, -1)
    bits = (bp.astype(np.int64)[:, None] >> shifts) & 1
    w = bits.reshape(OUT_F, IN_F).astype(np.float32) * 2 - 1
    return (x @ w.T).astype(np.float32)


def kernel(x: np.ndarray, bp: np.ndarray) -> np.ndarray:
    x = np.asarray(x, dtype=np.float32)
    bp = np.asarray(bp)
    # fp8 planes scale x by up to 2^5; |x| must stay below the TRN E4M3
    # max normal (240) / 32 = 7.5. Standard-normal inputs sit near 5.1.
    if (not np.isfinite(x).all()) or np.abs(x).max() >= 7.0 \
            or bp.min() < 0 or bp.max() > 255:
        return _host_reference(x, bp)
    out, _ = _run(x, bp)
    return out


if __name__ == "__main__":
    rng = np.random.default_rng(0)
    x = rng.standard_normal((TOKENS, IN_F), dtype=np.float32)
    bp = rng.integers(0, 256, (OUT_F * IN_F // 8,), dtype=np.int32)
    out = kernel(x, bp)
    ref = _host_reference(x, bp)
    rel = np.linalg.norm(out - ref) / np.linalg.norm(ref)
    print("self-check rel err:", rel)
